# revision 11
# baseline (speedup 1.0000x reference)
"""FFM layer kernel for Trainium2 (8 NeuronCores, data-parallel over batch).

Math (reference):
  idx[b,j]  = 13 + j*10000 + sparse_x[b,j]                 (26 sparse fields)
  linear    = dense_x @ w[:13] + sum_j w[idx] + w0         (B,1)
  field_f   = einsum('bd,dfk', dense_x, v[:13]) + sum_j v[idx]   (B,39,8)
  s         = sum_f field_f                                 (B,8)
  cross     = 0.5*(sum_k s^2 - sum_{f,k} field_f^2)
  out       = sigmoid(linear + cross)

Device strategy (per core, 512 samples): fp16 padded table rows (384 f16 =
768B) carrying [v row | w | s_row[k]=sum_f v[.,f,k]], 1024-idx pair-field
gathers (int16 idx spans two adjacent vocab blocks via +10000 offsets)
balanced 3328 rows per SWDGE queue, single_packet=False so SDMA drain
overlaps Q7 descriptor gen, four fp16 accumulator chains (one per queue)
plus an fp32 chain for the high-magnitude s columns, PE matmul for the
dense part, DVE/ACT final phase with s read from the precomputed columns.
"""

import os
import numpy as np

N_DENSE = 13
N_SPARSE = 26
VOCAB = 10000
N_FIELD = 39
N_FEAT = N_DENSE + N_SPARSE * VOCAB  # 260013
K = 8
ROW = N_FIELD * K  # 312 v elems; w at col 312; s_row at cols 313-320
USED = ROW + 1  # 313 cols accumulated in fp16 (v + w)
SCOL = ROW + 1  # first s column
ROWE = 384  # padded fp16 row -> 768 B (%256==0)
BATCH = 4096
N_CORES = 8
BC = BATCH // N_CORES  # 512 per core
P = 128
NCHUNK = BC // P  # 4
IDXC = BC // 16  # 32 int16 index columns per field
NQ = 4  # SWDGE queues / GPSIMD core pairs

# per-queue gather schedule: (first_field, n_fields, first_sample, n_idxs)
# each queue moves exactly 3328 rows; fields 6 and 19 split across queues
SCHED = [
    [(0, 2, 0, 1024), (2, 2, 0, 1024), (4, 2, 0, 1024), (6, 1, 0, 256)],
    [(7, 2, 0, 1024), (9, 2, 0, 1024), (11, 2, 0, 1024), (6, 1, 256, 256)],
    [(13, 2, 0, 1024), (15, 2, 0, 1024), (17, 2, 0, 1024), (19, 1, 0, 256)],
    [(20, 2, 0, 1024), (22, 2, 0, 1024), (24, 2, 0, 1024), (19, 1, 256, 256)],
]

_CACHE: dict = {}


def _build_program():
    import concourse.bacc as bacc
    import concourse.tile as tile
    import concourse.mybir as mybir

    f32 = mybir.dt.float32
    f16 = mybir.dt.float16
    i16 = mybir.dt.int16

    nc = bacc.Bacc(
        "TRN2", target_bir_lowering=False, debug=False, num_swdge_queues=NQ
    )

    table = nc.dram_tensor("table", [N_FEAT, ROWE], f16, kind="ExternalInput")
    xt = nc.dram_tensor("xt", [P, BC], f32, kind="ExternalInput")
    vd = nc.dram_tensor("vd", [P, ROWE], f32, kind="ExternalInput")
    idx16 = nc.dram_tensor("idx16", [P, N_SPARSE * IDXC], i16, kind="ExternalInput")
    # out[p, c] = sigmoid result for sample c*128+p; host transposes
    out = nc.dram_tensor("out", [P, NCHUNK], f32, kind="ExternalOutput")

    with tile.TileContext(nc) as tc:
        with (
            tc.tile_pool(name="const", bufs=1) as cpool,
            tc.tile_pool(name="gather", bufs=4 * NQ) as gpool,
            tc.tile_pool(name="work", bufs=1) as wpool,
            tc.tile_pool(name="psum", bufs=1, space="PSUM") as ppool,
        ):
            # index tiles first: the gathers depend only on these
            idx_sb = cpool.tile([P, N_SPARSE * IDXC], i16)
            nc.scalar.dma_start(out=idx_sb[:], in_=idx16[:])
            # dense inputs on the other HWDGE queue; only the PE needs them
            xt_sb = cpool.tile([P, BC], f32)
            nc.sync.dma_start(out=xt_sb[:], in_=xt[:])
            vd_sb = cpool.tile([P, ROWE], f32)
            nc.sync.dma_start(out=vd_sb[:], in_=vd[:])

            # preload the ACT sigmoid table off the critical tail
            warm = cpool.tile([P, 1], f32)
            nc.vector.memset(warm[:], 0.0)
            warm2 = cpool.tile([P, 1], f32)
            nc.scalar.activation(
                warm2[:], warm[:], mybir.ActivationFunctionType.Sigmoid
            )

            # dense part: one matmul per chunk, each into its own PSUM bank
            psum = ppool.tile([P, NCHUNK, 512], f32, space="PSUM")
            for c in range(NCHUNK):
                nc.tensor.matmul(
                    out=psum[:, c, :ROWE],
                    lhsT=xt_sb[:, c * P:(c + 1) * P],
                    rhs=vd_sb[:],
                    start=True,
                    stop=True,
                )

            nreg = {
                1024: nc.gpsimd.to_reg(1024),
                256: nc.gpsimd.to_reg(256),
            }
            # emit gathers round-robin across queues; the first gather also
            # absorbs the one-time ~6-9us GPSIMD ext-isa IRAM load.
            # single_packet=False: per-descriptor doorbells let the SDMA
            # drain stream behind Q7 descriptor generation.
            gathers = []  # (tile, queue, n_fields, first_chunk, chunks/field)
            for r in range(4):
                for q in range(NQ):
                    f0, nf, s0, ni = SCHED[q][r]
                    nch = ni // P
                    g = gpool.tile([P, nch, ROWE], f16, tag="g", name=f"g{q}_{r}")
                    base = N_DENSE + f0 * VOCAB
                    col0 = f0 * IDXC + s0 // 16
                    nc.gpsimd.dma_gather(
                        out_ap=g[:],
                        in_ap=table[base:base + nf * VOCAB, :],
                        idxs_ap=idx_sb[:, col0:col0 + ni // 16],
                        num_idxs=ni,
                        num_idxs_reg=nreg[ni],
                        elem_size=ROWE,
                        single_packet=False,
                        queue_num=q,
                    )
                    gathers.append((g, q, nf, s0 // P, nch // nf))

            # four fp16 accumulator chains (one per queue, like-magnitude
            # partials keep fp16 rounding at the baseline level) plus an
            # fp32 chain for the high-magnitude s columns (313-320).
            # Every queue's first gather is a pair tile, which initializes
            # its chain with a single two-slice add.
            accs = [wpool.tile([P, NCHUNK, USED + 1], f16, name=f"acc{q}")
                    for q in range(NQ)]
            # sacc2[p, h*4+c, k]: two parallel fp32 s-chains, flat over the
            # pair-tile chunk axis so each tile needs ONE small add
            sacc2 = wpool.tile([P, 2 * NCHUNK, K], f32)
            inited = [False] * NQ
            sinit = False
            for g, q, nf, c0, cpf in gathers:
                # fp16 main accumulation (cols 0:314)
                if not inited[q]:
                    assert nf == 2 and c0 == 0
                    nc.vector.tensor_tensor(
                        out=accs[q][:],
                        in0=g[:, 0:cpf, :USED + 1],
                        in1=g[:, cpf:2 * cpf, :USED + 1],
                        op=mybir.AluOpType.add,
                    )
                    inited[q] = True
                else:
                    for h in range(nf):
                        dst = accs[q][:, c0:c0 + cpf, :]
                        nc.vector.tensor_tensor(
                            out=dst, in0=dst,
                            in1=g[:, h * cpf:(h + 1) * cpf, :USED + 1],
                            op=mybir.AluOpType.add,
                        )
                # fp32 s-column accumulation, one op per tile
                ssrc = g[:, :, SCOL:SCOL + K]  # [P, nf*cpf, K]
                sdst = sacc2[:, c0:c0 + nf * cpf, :]
                if not sinit:
                    nc.vector.tensor_copy(out=sdst, in_=ssrc)
                    sinit = True
                else:
                    nc.vector.tensor_tensor(
                        out=sdst, in0=sdst, in1=ssrc,
                        op=mybir.AluOpType.add,
                    )

            # combine the four chains (c01/c23 are independent -> pipelined)
            nc.vector.tensor_tensor(out=accs[0][:], in0=accs[0][:],
                                    in1=accs[1][:], op=mybir.AluOpType.add)
            nc.vector.tensor_tensor(out=accs[2][:], in0=accs[2][:],
                                    in1=accs[3][:], op=mybir.AluOpType.add)
            nc.vector.tensor_tensor(out=accs[0][:], in0=accs[0][:],
                                    in1=accs[2][:], op=mybir.AluOpType.add)

            # field[p, c, 0:313] = psum + acc   (col 312 = full linear term)
            field = wpool.tile([P, NCHUNK, 320], f32)
            nc.vector.tensor_tensor(
                out=field[:, :, :USED], in0=psum[:, :, :USED],
                in1=accs[0][:, :, :USED], op=mybir.AluOpType.add,
            )

            # s = dense part (psum cols 313-320, from vd s columns) + both
            # halves of the flat fp32 s accumulator
            s_t = wpool.tile([P, NCHUNK, K], f32)
            nc.vector.tensor_tensor(
                out=s_t[:], in0=sacc2[:, 0:NCHUNK, :],
                in1=sacc2[:, NCHUNK:2 * NCHUNK, :], op=mybir.AluOpType.add,
            )
            s = wpool.tile([P, NCHUNK, K], f32)
            nc.vector.tensor_tensor(
                out=s[:], in0=psum[:, :, SCOL:SCOL + K], in1=s_t[:],
                op=mybir.AluOpType.add,
            )

            # q = sum(field[:, :, :312]^2) per chunk
            sq = wpool.tile([P, NCHUNK, ROW], f32)
            nc.scalar.square(sq[:], field[:, :, :ROW])
            qs = wpool.tile([P, NCHUNK, 1], f32)
            nc.vector.reduce_sum(out=qs[:], in_=sq[:], axis=mybir.AxisListType.X)

            ss = wpool.tile([P, NCHUNK, K], f32)
            nc.vector.tensor_tensor(out=ss[:], in0=s[:], in1=s[:],
                                    op=mybir.AluOpType.mult)
            ssum = wpool.tile([P, NCHUNK, 1], f32)
            nc.vector.reduce_sum(out=ssum[:], in_=ss[:], axis=mybir.AxisListType.X)
            d = wpool.tile([P, NCHUNK, 1], f32)
            nc.vector.tensor_tensor(out=d[:], in0=ssum[:], in1=qs[:],
                                    op=mybir.AluOpType.subtract)
            # dd = 0.5*d + linear
            dd = wpool.tile([P, NCHUNK, 1], f32)
            nc.vector.scalar_tensor_tensor(
                out=dd[:], in0=d[:], scalar=0.5, in1=field[:, :, ROW:ROW + 1],
                op0=mybir.AluOpType.mult, op1=mybir.AluOpType.add,
            )
            oc = wpool.tile([P, NCHUNK], f32)
            nc.scalar.activation(
                oc[:], dd[:, :, 0], mybir.ActivationFunctionType.Sigmoid
            )
            nc.sync.dma_start(out=out[:], in_=oc[:])

    nc.compile()
    return nc


def _prep_inputs(dense_x, sparse_x, w0, w, v):
    table = np.zeros((N_FEAT, ROWE), dtype=np.float16)
    table[:, :ROW] = v.reshape(N_FEAT, ROW).astype(np.float16)
    table[:, ROW] = w[:, 0].astype(np.float16)
    # per-k field sums: s_row[i, k] = sum_f v[i, f, k]
    table[:, SCOL:SCOL + K] = v.sum(axis=1).astype(np.float16)

    vd = np.zeros((P, ROWE), dtype=np.float32)
    vd[:N_DENSE, :ROW] = v[:N_DENSE].reshape(N_DENSE, ROW)
    vd[:N_DENSE, ROW] = w[:N_DENSE, 0]
    vd[N_DENSE, ROW] = np.float32(w0[0])
    vd[:N_DENSE, SCOL:SCOL + K] = v[:N_DENSE].sum(axis=1)

    xt_full = np.zeros((P, BATCH), dtype=np.float32)
    xt_full[:N_DENSE] = dense_x.T
    xt_full[N_DENSE] = 1.0

    # idx offsets: field f gathers from a window starting at its pair/quarter
    # window base; value offset = (f - window_first_field) * VOCAB
    off = np.zeros(N_SPARSE, dtype=np.int16)
    for qsched in SCHED:
        for f0, nf, _s0, _ni in qsched:
            for h in range(nf):
                off[f0 + h] = h * VOCAB

    in_maps = []
    for r in range(N_CORES):
        b0 = r * BC
        sp = sparse_x[b0:b0 + BC].astype(np.int16)  # values < 10000 fit
        idx16 = np.zeros((P, N_SPARSE * IDXC), dtype=np.int16)
        for j in range(N_SPARSE):
            blk = (sp[:, j] + off[j]).reshape(IDXC, 16).T
            idx16[:, j * IDXC:(j + 1) * IDXC] = np.tile(blk, (P // 16, 1))
        in_maps.append(
            {
                "table": table,
                "xt": np.ascontiguousarray(xt_full[:, b0:b0 + BC]),
                "vd": vd,
                "idx16": idx16,
            }
        )
    return in_maps


def kernel(dense_x, sparse_x, w0, w, v, _trace=False, _trace_kwargs=None):
    from concourse.bass_utils import run_bass_kernel_spmd

    if "nc" not in _CACHE:
        _CACHE["nc"] = _build_program()
    nc = _CACHE["nc"]

    in_maps = _prep_inputs(dense_x, sparse_x, w0, w, v)
    kw = {}
    if _trace:
        kw["trace"] = True
        if _trace_kwargs:
            kw.update(_trace_kwargs)
    res = run_bass_kernel_spmd(nc, in_maps, core_ids=list(range(N_CORES)), **kw)
    # device out[p, c] holds sample c*128+p of the core's 512-sample slice
    outs = [res.results[r]["out"].T.reshape(BC, 1) for r in range(N_CORES)]
    full = np.concatenate(outs, axis=0).astype(np.float32)
    if _trace:
        _CACHE["last_exec_time_ns"] = res.exec_time_ns
        _CACHE["last_results"] = res
    return full


# revision 13
# speedup vs baseline: 1.1661x; 1.1661x over previous
"""FFM layer kernel for Trainium2 (8 NeuronCores, data-parallel over batch).

Math (reference):
  idx[b,j]  = 13 + j*10000 + sparse_x[b,j]                 (26 sparse fields)
  linear    = dense_x @ w[:13] + sum_j w[idx] + w0         (B,1)
  field_f   = einsum('bd,dfk', dense_x, v[:13]) + sum_j v[idx]   (B,39,8)
  s         = sum_f field_f                                 (B,8)
  cross     = 0.5*(sum_k s^2 - sum_{f,k} field_f^2)
  out       = sigmoid(linear + cross)

Device strategy (per core, 512 samples): fp16 padded table rows (384 f16 =
768B) carrying [v row | w | s_row[k]=sum_f v[.,f,k]], 1024-idx pair-field
gathers (int16 idx spans two adjacent vocab blocks via +10000 offsets)
balanced 3328 rows per SWDGE queue, single_packet=False so SDMA drain
overlaps Q7 descriptor gen, four fp16 accumulator chains (one per queue)
plus an fp32 chain for the high-magnitude s columns, PE matmul for the
dense part, DVE/ACT final phase with s read from the precomputed columns.
"""

import os
import numpy as np

N_DENSE = 13
N_SPARSE = 26
VOCAB = 10000
N_FIELD = 39
N_FEAT = N_DENSE + N_SPARSE * VOCAB  # 260013
K = 8
ROW = N_FIELD * K  # 312 v elems; w at col 312; s_row at cols 313-320
USED = ROW + 1  # 313 cols accumulated in fp16 (v + w)
SCOL = ROW + 1  # first s column
ROWE = 384  # padded fp16 row -> 768 B (%256==0)
BATCH = 4096
N_CORES = 8
BC = BATCH // N_CORES  # 512 per core
P = 128
NCHUNK = BC // P  # 4
IDXC = BC // 16  # 32 int16 index columns per field
NQ = 4  # SWDGE queues / GPSIMD core pairs

# per-queue gather schedule: (first_field, n_fields, first_sample, n_idxs)
# each queue moves exactly 3328 rows; fields 6 and 19 split across queues
SCHED = [
    [(0, 2, 0, 1024), (2, 2, 0, 1024), (4, 2, 0, 1024), (6, 1, 0, 256)],
    [(7, 2, 0, 1024), (9, 2, 0, 1024), (11, 2, 0, 1024), (6, 1, 256, 256)],
    [(13, 2, 0, 1024), (15, 2, 0, 1024), (17, 2, 0, 1024), (19, 1, 0, 256)],
    [(20, 2, 0, 1024), (22, 2, 0, 1024), (24, 2, 0, 1024), (19, 1, 256, 256)],
]

_CACHE: dict = {}


def _build_program():
    import concourse.bacc as bacc
    import concourse.tile as tile
    import concourse.mybir as mybir

    f32 = mybir.dt.float32
    f16 = mybir.dt.float16
    i16 = mybir.dt.int16

    nc = bacc.Bacc(
        "TRN2", target_bir_lowering=False, debug=False, num_swdge_queues=NQ
    )

    table = nc.dram_tensor("table", [N_FEAT, ROWE], f16, kind="ExternalInput")
    xt = nc.dram_tensor("xt", [P, BC], f32, kind="ExternalInput")
    vd = nc.dram_tensor("vd", [P, ROWE], f32, kind="ExternalInput")
    idx16 = nc.dram_tensor("idx16", [P, N_SPARSE * IDXC], i16, kind="ExternalInput")
    # out[p, c] = sigmoid result for sample c*128+p; host transposes
    out = nc.dram_tensor("out", [P, NCHUNK], f32, kind="ExternalOutput")

    with tile.TileContext(nc) as tc:
        with (
            tc.tile_pool(name="const", bufs=1) as cpool,
            tc.tile_pool(name="gather", bufs=4 * NQ) as gpool,
            tc.tile_pool(name="work", bufs=1) as wpool,
            tc.tile_pool(name="psum", bufs=1, space="PSUM") as ppool,
        ):
            # index tiles first: the gathers depend only on these
            idx_sb = cpool.tile([P, N_SPARSE * IDXC], i16)
            nc.scalar.dma_start(out=idx_sb[:], in_=idx16[:])
            # dense inputs on the other HWDGE queue; only the PE needs them
            xt_sb = cpool.tile([P, BC], f32)
            nc.sync.dma_start(out=xt_sb[:], in_=xt[:])
            vd_sb = cpool.tile([P, ROWE], f32)
            nc.sync.dma_start(out=vd_sb[:], in_=vd[:])

            # preload the ACT sigmoid table off the critical tail
            warm = cpool.tile([P, 1], f32)
            nc.vector.memset(warm[:], 0.0)
            warm2 = cpool.tile([P, 1], f32)
            nc.scalar.activation(
                warm2[:], warm[:], mybir.ActivationFunctionType.Sigmoid
            )

            # dense part: one matmul per chunk, each into its own PSUM bank
            psum = ppool.tile([P, NCHUNK, 512], f32, space="PSUM")
            for c in range(NCHUNK):
                nc.tensor.matmul(
                    out=psum[:, c, :ROWE],
                    lhsT=xt_sb[:, c * P:(c + 1) * P],
                    rhs=vd_sb[:],
                    start=True,
                    stop=True,
                )

            nreg = {
                1024: nc.gpsimd.to_reg(1024),
                256: nc.gpsimd.to_reg(256),
            }
            # emit gathers round-robin across queues; the first gather also
            # absorbs the one-time ~6-9us GPSIMD ext-isa IRAM load.
            # single_packet=False: per-descriptor doorbells let the SDMA
            # drain stream behind Q7 descriptor generation.
            gathers = []  # (tile, queue, n_fields, first_chunk, chunks/field)
            for r in range(4):
                for q in range(NQ):
                    f0, nf, s0, ni = SCHED[q][r]
                    nch = ni // P
                    g = gpool.tile([P, nch, ROWE], f16, tag="g", name=f"g{q}_{r}")
                    base = N_DENSE + f0 * VOCAB
                    col0 = f0 * IDXC + s0 // 16
                    nc.gpsimd.dma_gather(
                        out_ap=g[:],
                        in_ap=table[base:base + nf * VOCAB, :],
                        idxs_ap=idx_sb[:, col0:col0 + ni // 16],
                        num_idxs=ni,
                        num_idxs_reg=nreg[ni],
                        elem_size=ROWE,
                        single_packet=True,
                        queue_num=q,
                    )
                    gathers.append((g, q, nf, s0 // P, nch // nf))

            # two fp16 accumulator chains with alternating slice assignment:
            # adjacent DVE adds hit different chains so they pipeline (~875ns
            # cadence vs ~1.5us for a dependent chain).  An fp32 chain takes
            # the high-magnitude s columns (313-320), one flat add per tile.
            accs = [wpool.tile([P, NCHUNK, USED + 1], f16, name=f"acc{i}")
                    for i in range(2)]
            # sacc2[p, h*4+c, k]: two parallel fp32 s-chains, flat over the
            # pair-tile chunk axis
            sacc2 = wpool.tile([P, 2 * NCHUNK, K], f32)
            # init: chain i <- field slice i of the first two pair tiles
            g0, g1 = gathers[0][0], gathers[1][0]
            for i in range(2):
                nc.vector.tensor_tensor(
                    out=accs[i][:],
                    in0=g0[:, i * NCHUNK:(i + 1) * NCHUNK, :USED + 1],
                    in1=g1[:, i * NCHUNK:(i + 1) * NCHUNK, :USED + 1],
                    op=mybir.AluOpType.add,
                )
            for t, (g, q, nf, c0, cpf) in enumerate(gathers):
                if t >= 2:
                    for h in range(nf):
                        ch = accs[(t * 2 + h) % 2]
                        dst = ch[:, c0:c0 + cpf, :]
                        nc.vector.tensor_tensor(
                            out=dst, in0=dst,
                            in1=g[:, h * cpf:(h + 1) * cpf, :USED + 1],
                            op=mybir.AluOpType.add,
                        )
                # fp32 s-column accumulation, one op per tile
                ssrc = g[:, :, SCOL:SCOL + K]  # [P, nf*cpf, K]
                sdst = sacc2[:, c0:c0 + nf * cpf, :]
                if t == 0:
                    nc.vector.tensor_copy(out=sdst, in_=ssrc)
                else:
                    nc.vector.tensor_tensor(
                        out=sdst, in0=sdst, in1=ssrc,
                        op=mybir.AluOpType.add,
                    )

            # combine the two chains
            nc.vector.tensor_tensor(out=accs[0][:], in0=accs[0][:],
                                    in1=accs[1][:], op=mybir.AluOpType.add)

            # field[p, c, 0:313] = psum + acc   (col 312 = full linear term)
            field = wpool.tile([P, NCHUNK, 320], f32)
            nc.vector.tensor_tensor(
                out=field[:, :, :USED], in0=psum[:, :, :USED],
                in1=accs[0][:, :, :USED], op=mybir.AluOpType.add,
            )

            # s = dense part (psum cols 313-320, from vd s columns) + both
            # halves of the flat fp32 s accumulator
            s_t = wpool.tile([P, NCHUNK, K], f32)
            nc.vector.tensor_tensor(
                out=s_t[:], in0=sacc2[:, 0:NCHUNK, :],
                in1=sacc2[:, NCHUNK:2 * NCHUNK, :], op=mybir.AluOpType.add,
            )
            s = wpool.tile([P, NCHUNK, K], f32)
            nc.vector.tensor_tensor(
                out=s[:], in0=psum[:, :, SCOL:SCOL + K], in1=s_t[:],
                op=mybir.AluOpType.add,
            )

            # q = sum(field[:, :, :312]^2) per chunk
            sq = wpool.tile([P, NCHUNK, ROW], f32)
            nc.scalar.square(sq[:], field[:, :, :ROW])
            qs = wpool.tile([P, NCHUNK, 1], f32)
            nc.vector.reduce_sum(out=qs[:], in_=sq[:], axis=mybir.AxisListType.X)

            ss = wpool.tile([P, NCHUNK, K], f32)
            nc.vector.tensor_tensor(out=ss[:], in0=s[:], in1=s[:],
                                    op=mybir.AluOpType.mult)
            ssum = wpool.tile([P, NCHUNK, 1], f32)
            nc.vector.reduce_sum(out=ssum[:], in_=ss[:], axis=mybir.AxisListType.X)
            d = wpool.tile([P, NCHUNK, 1], f32)
            nc.vector.tensor_tensor(out=d[:], in0=ssum[:], in1=qs[:],
                                    op=mybir.AluOpType.subtract)
            # dd = 0.5*d + linear
            dd = wpool.tile([P, NCHUNK, 1], f32)
            nc.vector.scalar_tensor_tensor(
                out=dd[:], in0=d[:], scalar=0.5, in1=field[:, :, ROW:ROW + 1],
                op0=mybir.AluOpType.mult, op1=mybir.AluOpType.add,
            )
            oc = wpool.tile([P, NCHUNK], f32)
            nc.scalar.activation(
                oc[:], dd[:, :, 0], mybir.ActivationFunctionType.Sigmoid
            )
            nc.sync.dma_start(out=out[:], in_=oc[:])

    nc.compile()
    return nc


def _prep_inputs(dense_x, sparse_x, w0, w, v):
    table = np.zeros((N_FEAT, ROWE), dtype=np.float16)
    table[:, :ROW] = v.reshape(N_FEAT, ROW).astype(np.float16)
    table[:, ROW] = w[:, 0].astype(np.float16)
    # per-k field sums: s_row[i, k] = sum_f v[i, f, k]
    table[:, SCOL:SCOL + K] = v.sum(axis=1).astype(np.float16)

    vd = np.zeros((P, ROWE), dtype=np.float32)
    vd[:N_DENSE, :ROW] = v[:N_DENSE].reshape(N_DENSE, ROW)
    vd[:N_DENSE, ROW] = w[:N_DENSE, 0]
    vd[N_DENSE, ROW] = np.float32(w0[0])
    vd[:N_DENSE, SCOL:SCOL + K] = v[:N_DENSE].sum(axis=1)

    xt_full = np.zeros((P, BATCH), dtype=np.float32)
    xt_full[:N_DENSE] = dense_x.T
    xt_full[N_DENSE] = 1.0

    # idx offsets: field f gathers from a window starting at its pair/quarter
    # window base; value offset = (f - window_first_field) * VOCAB
    off = np.zeros(N_SPARSE, dtype=np.int16)
    for qsched in SCHED:
        for f0, nf, _s0, _ni in qsched:
            for h in range(nf):
                off[f0 + h] = h * VOCAB

    in_maps = []
    for r in range(N_CORES):
        b0 = r * BC
        sp = sparse_x[b0:b0 + BC].astype(np.int16)  # values < 10000 fit
        idx16 = np.zeros((P, N_SPARSE * IDXC), dtype=np.int16)
        for j in range(N_SPARSE):
            blk = (sp[:, j] + off[j]).reshape(IDXC, 16).T
            idx16[:, j * IDXC:(j + 1) * IDXC] = np.tile(blk, (P // 16, 1))
        in_maps.append(
            {
                "table": table,
                "xt": np.ascontiguousarray(xt_full[:, b0:b0 + BC]),
                "vd": vd,
                "idx16": idx16,
            }
        )
    return in_maps


def kernel(dense_x, sparse_x, w0, w, v, _trace=False, _trace_kwargs=None):
    from concourse.bass_utils import run_bass_kernel_spmd

    if "nc" not in _CACHE:
        _CACHE["nc"] = _build_program()
    nc = _CACHE["nc"]

    in_maps = _prep_inputs(dense_x, sparse_x, w0, w, v)
    kw = {}
    if _trace:
        kw["trace"] = True
        if _trace_kwargs:
            kw.update(_trace_kwargs)
    res = run_bass_kernel_spmd(nc, in_maps, core_ids=list(range(N_CORES)), **kw)
    # device out[p, c] holds sample c*128+p of the core's 512-sample slice
    outs = [res.results[r]["out"].T.reshape(BC, 1) for r in range(N_CORES)]
    full = np.concatenate(outs, axis=0).astype(np.float32)
    if _trace:
        _CACHE["last_exec_time_ns"] = res.exec_time_ns
        _CACHE["last_results"] = res
    return full
